# revision 16
# baseline (speedup 1.0000x reference)
"""CRF negative log-likelihood loss on 8 Trainium2 NeuronCores.

Strategy (v5)
-------------
Data-parallel over batch: 1024 sequences -> 8 cores x 128.

The log-partition (forward algorithm) is a T=512-step linear recurrence in
the exp domain:  alpha_t = ehat_t * (M~^T alpha_{t-1}),  with
M~ = exp(-MU)*exp(trans) folded into the stationary matmul weights (MU keeps
magnitudes bounded, restored on the host as +511*MU).

The sequence is split into S=16 overlapped chains; each warms up DELTA
steps before its 32-step window.  Chain 0 is injected with the exact
alpha_0; chain 15 is shifted to end exactly at t=511.  Per-window growth
factors are recovered on the host from raw state snapshots.

Layout: 16 chains packed 2-high (96 partitions) x 4 independent column
groups of 256 (4 chains each).

v5 changes vs v4 (baseline 61.9us):
- All slab DMA on the two HWDGE rings (sync + scalar) instead of SWDGE:
  kills the ~2.3us-per-dma_start Q7 descriptor DRAIN; first chunk lands
  ~1.5us instead of ~10us.
- Weights padded to [96,128] so the compiler's fast-weight-load engages
  (LDWEIGHTS 4x, fully hidden behind the previous matmul stream).
- PE prewarmed with dummy matmuls during the DMA fill (HAM clock ramp).
- Per-round mul/evac rebalanced: 3 groups multiplied directly out of PSUM
  by DVE (1x mode, unavoidable with fp32 PSUM), 1 rotating group handled
  by GpSimd directly from PSUM; the rotation keeps each chain's slow GS
  edge to every 4th round (state pool bufs give slack).
- Output DMA split: snapshots ship right after round DELTA+1, finals at
  the end (shorter tail).
"""

import os
import sys

sys.path.insert(0, "/opt/trn_rl_repo")

import numpy as np
import ml_dtypes

import concourse.bass as bass
import concourse.bacc as bacc
import concourse.mybir as mybir
from concourse import tile
from concourse import bass_utils

if bool(int(os.environ.get("CRF_LDWOPT", "0"))) and not getattr(
    bass_utils, "_crf_ldwopt_patch", False
):
    # Let walrus double-buffer LDWEIGHTS (elides the ~120ns reload that
    # serializes with every matmul; all our matmuls share one weight tile).
    _orig_run_command = bass_utils.run_command

    def _run_command_ldwopt(argv, **kwargs):
        argv = [
            "--enable-ldw-opt=true" if a == "--enable-ldw-opt=false" else a
            for a in argv
        ]
        return _orig_run_command(argv, **kwargs)

    bass_utils.run_command = _run_command_ldwopt
    bass_utils._crf_ldwopt_patch = True

BF16 = ml_dtypes.bfloat16

B, T, K = 1024, 512, 48
NCORES = 8
BL = B // NCORES          # 128 sequences per core
S = 16                    # chains
DELTA = int(os.environ.get("CRF_DELTA", "2"))
R = DELTA + 32
MU = 4.4                  # growth prescale folded into weights
NG = 4                    # independent column groups
GF = 256                  # free-dim per group tile (2 chains x 128)
P2 = 2 * K                # 96 partitions (2 chains stacked)
WPAD = int(os.environ.get("CRF_WPAD", "96"))
PREWARM = int(os.environ.get("CRF_PREWARM", "0"))
GS_DIRECT = bool(int(os.environ.get("CRF_GS_DIRECT", "1")))

# Rounds per DMA chunk; small leading chunks so early rounds never starve.
_BASE_CHUNKS = [1, 2, 4, 6, 9]
CHUNKS = list(_BASE_CHUNKS) + [R - sum(_BASE_CHUNKS)]
assert CHUNKS[-1] > 0
_R2C = {}
_acc = 0
for _i, _c in enumerate(CHUNKS):
    for _j in range(_c):
        _R2C[_acc + _j + 1] = (_i, _j)
    _acc += _c
_CSTART = np.cumsum([0] + CHUNKS[:-1])

_cache = {}


def _chain_t0():
    t0 = np.array([32 * c - DELTA for c in range(S)], np.int64)
    t0[S - 1] = (T - 1) - R
    return t0


_ROLE_CYCLE = os.environ.get("CRF_ROLES", "ddab")


def _role(r, g):
    """Mul path for (round, group): 'd' = DVE direct from PSUM,
    'a' = ACT evac copy + DVE 2x mul, 'b' = ACT evac copy + GpSimd mul."""
    if r <= 2 or r == R:
        return "d"
    return _ROLE_CYCLE[(r + g) % len(_ROLE_CYCLE)]


def _build_program():
    nc = bacc.Bacc(
        "TRN2",
        debug=False,
        enable_asserts=False,
        target_bir_lowering=False,
        num_devices=NCORES,
    )
    f32 = mybir.dt.float32
    bf16 = mybir.dt.bfloat16

    slabs = [
        nc.dram_tensor(f"slab{h}", [P2, R * 2 * GF], bf16, kind="ExternalInput")
        for h in range(2)
    ]
    # wblk bf16 [P2,WPAD] | expstart f32 (rows 0:48) | vinit f32 [P2,1]
    consts = nc.dram_tensor(
        "consts", [P2, 2 * WPAD + 8], mybir.dt.int8, kind="ExternalInput"
    )

    out_snap = nc.dram_tensor("snaps", [P2, 5 * GF], bf16, kind="ExternalOutput")
    out_fin = nc.dram_tensor("fins", [P2, NG * GF], bf16, kind="ExternalOutput")

    def eh_slice(ehat, r, g):
        """ehat slice [P2, GF] for round r (1-based), group g."""
        i, j = _R2C[r]
        off = j * 2 * GF + (g % 2) * GF
        return ehat[g // 2][i][:, off : off + GF]

    with tile.TileContext(nc) as tc:
        with (
            tc.tile_pool(name="const", bufs=1) as const_pool,
            tc.tile_pool(name="ehat", bufs=1) as ehat_pool,
            tc.tile_pool(name="state", bufs=4) as state_pool,
            tc.tile_pool(name="evac", bufs=2) as evac_pool,
            tc.tile_pool(name="psum", bufs=1, space="PSUM") as psum_pool,
        ):
            consts_tile = const_pool.tile(
                [P2, 2 * WPAD + 8], mybir.dt.int8, tag="consts"
            )
            w_tile = consts_tile[:, 0 : 2 * WPAD].bitcast(bf16)     # [P2, WPAD]
            es_tile = consts_tile[0:K, 2 * WPAD : 2 * WPAD + 4].bitcast(f32)
            vi_tile = consts_tile[:, 2 * WPAD + 4 : 2 * WPAD + 8].bitcast(f32)
            warm_in = const_pool.tile([P2, GF], bf16, tag="warm")
            prime = const_pool.tile([K, 1], f32, tag="prime")

            with tc.high_priority():
                nc.sync.dma_start(consts_tile[:], consts.ap()[:])
                nc.vector.memset(warm_in[:], 0.0)
                nc.vector.memset(prime[:], 0.0)
                # Pull the one-time ACT table load into the DMA shadow.
                nc.scalar.copy(prime[:], prime[:])

            # PSUM tiles (128 partitions for the FWL-padded weights).
            ps_tiles = [
                psum_pool.tile([WPAD, GF], f32, tag=f"ps{g}", name=f"ps{g}")
                for g in range(NG)
            ]
            ps_warm = psum_pool.tile([WPAD, GF], f32, tag="pswarm", name="pswarm")

            # PE prewarm: dummy matmuls while DMA fills (HAM ramp).
            for i in range(PREWARM):
                nc.tensor.matmul(
                    ps_warm[:], w_tile[:], warm_in[:], start=True, stop=True
                )

            # Stream bf16 ehat slabs into residency via the two HWDGE rings.
            # Slab0 entirely from the (otherwise idle) sync ring in the
            # prologue; slab1 from the scalar ring, with late chunks issued
            # inside the round loop so they never clog the ACT sequencer
            # ahead of evac copies.
            ehat = [[None] * len(CHUNKS) for _ in range(2)]

            def issue_chunk(h, i):
                csz = CHUNKS[i]
                c0 = int(_CSTART[i]) * 2 * GF
                eh = ehat_pool.tile(
                    [P2, csz * 2 * GF], bf16, tag=f"eh{h}_{i}", bufs=1
                )
                eng = nc.sync if h == 0 else nc.scalar
                eng.dma_start(eh[:], slabs[h].ap()[:, c0 : c0 + csz * 2 * GF])
                ehat[h][i] = eh

            for i in range(len(CHUNKS)):
                issue_chunk(0, i)
            for i in range(2):
                issue_chunk(1, i)

            # Snapshot/final staging (muls write straight into slices).
            stage_s = const_pool.tile([P2, 5 * GF], bf16, tag="stage_s")
            stage_f = const_pool.tile([P2, NG * GF], bf16, tag="stage_f")

            def stage_slot(r, g):
                if r == DELTA:
                    return stage_s[:, g * GF : (g + 1) * GF]
                if r == DELTA + 1 and g == NG - 1:
                    return stage_s[:, NG * GF : (NG + 1) * GF]
                if r == R:
                    return stage_f[:, g * GF : (g + 1) * GF]
                return None

            def round_epilogue(r):
                if r == DELTA:
                    # Inject exact alpha_0 into chain 0 (group 0).
                    nc.vector.tensor_scalar_mul(
                        state[0][0:K, 0:BL],
                        eh_slice(ehat, r, 0)[0:K, 0:BL],
                        es_tile[:],
                    )
                if r == DELTA + 1:
                    nc.sync.dma_start(out_snap.ap()[:], stage_s[:])
                # Late slab1 chunks: issue ~3 rounds before their data is
                # needed, from inside the loop (keeps the ACT ring shallow).
                for i in range(2, len(CHUNKS)):
                    if r == max(2, int(_CSTART[i]) - 3):
                        issue_chunk(1, i)

            # Round 1 without matmul: alpha_1 = ehat_1 * (M~^T 1).
            state = []
            for g in range(NG):
                st = stage_slot(1, g)
                if st is None:
                    st = state_pool.tile(
                        [P2, GF], bf16, tag=f"st{g}", name=f"st{g}_1"
                    )[:]
                nc.vector.tensor_scalar_mul(
                    st, eh_slice(ehat, 1, g), vi_tile[:]
                )
                state.append(st)
            round_epilogue(1)

            for r in range(2, R + 1):
                # Evac'd (slow-path) groups' matmuls first.
                order = sorted(range(NG), key=lambda g: (_role(r, g) == "d",))
                for g in order:
                    ps = ps_tiles[g]
                    nc.tensor.matmul(
                        ps[:], w_tile[:], state[g], start=True, stop=True
                    )
                    st_new = stage_slot(r, g)
                    if st_new is None:
                        st_new = state_pool.tile(
                            [P2, GF], bf16, tag=f"st{g}", name=f"st{g}_{r}"
                        )[:]
                    role = _role(r, g)
                    if role in ("a", "b"):
                        ut = evac_pool.tile(
                            [P2, GF], bf16, tag=f"u{g % 2}", name=f"u{g}_{r}"
                        )
                        nc.scalar.copy(ut[:], ps[0:P2, :])
                        mul_eng = nc.gpsimd if role == "b" else nc.vector
                        mul_eng.tensor_mul(
                            st_new, ut[:], eh_slice(ehat, r, g)
                        )
                    else:
                        nc.vector.tensor_mul(
                            st_new, ps[0:P2, :], eh_slice(ehat, r, g)
                        )
                    state[g] = st_new

                round_epilogue(r)

            nc.sync.dma_start(out_fin.ap()[:], stage_f[:])
    nc.compile()
    return nc


def _host_slabs(eh_local):
    """eh_local: [BL, T, K] fp32 ehat -> list of 2 slabs [P2, R*2*GF] bf16."""
    et = np.ascontiguousarray(eh_local.transpose(1, 2, 0))  # [T, K, BL]
    slab = np.ones((2, 2, K, R, 4, BL), np.float32)  # [h, p, k, r, q, b]
    t0 = _chain_t0()
    rr = np.arange(1, R + 1)
    for c in range(S):
        h, q, p = c // 8, (c % 8) // 2, c % 2
        ts = t0[c] + rr
        valid = np.nonzero(ts >= 0)[0]
        slab[h, p, :, valid, q, :] = et[ts[valid]]
    return [
        np.ascontiguousarray(slab[h].reshape(P2, R * 4 * BL)).astype(BF16)
        for h in range(2)
    ]


def _gold_score(emissions, tags, mask, transitions, start_transitions, end_transitions):
    em = np.asarray(emissions, np.float32)
    tg = np.asarray(tags, np.int64)
    mk = np.asarray(mask, bool)
    emit = np.take_along_axis(em, tg[..., None], axis=2)[..., 0]
    tr = np.asarray(transitions, np.float32)[tg[:, :-1], tg[:, 1:]]
    mf = mk[:, 1:].astype(np.float32)
    score = (
        np.asarray(start_transitions, np.float32)[tg[:, 0]]
        + emit[:, 0]
        + ((tr + emit[:, 1:]) * mf).sum(axis=1)
    )
    lengths = mk.astype(np.int64).sum(axis=1) - 1
    last = np.take_along_axis(tg, lengths[:, None], axis=1)[:, 0]
    return score + np.asarray(end_transitions, np.float32)[last]


def kernel(emissions, tags, mask, transitions, start_transitions, end_transitions):
    em = np.asarray(emissions, np.float32)
    trans = np.asarray(transitions, np.float32)
    start = np.asarray(start_transitions, np.float32)
    end = np.asarray(end_transitions, np.float32)

    if "nc" not in _cache:
        _cache["nc"] = _build_program()
    nc = _cache["nc"]

    mt = (np.exp(-MU) * np.exp(trans)).astype(np.float32)  # [K,K] prescaled
    wblk = np.zeros((P2, WPAD), np.float32)
    wblk[:K, :K] = mt
    wblk[K:, K : 2 * K] = mt
    wblk = wblk.astype(BF16)
    es = np.exp(start).astype(np.float32)
    vi = mt.sum(axis=0).astype(np.float32)  # (M~^T 1)_i = sum_j mt[j,i]
    vinit = np.concatenate([vi, vi]).reshape(P2, 1)

    consts = np.zeros((P2, 2 * WPAD + 8), np.int8)
    consts[:, 0 : 2 * WPAD] = wblk.view(np.int8).reshape(P2, 2 * WPAD)
    consts[:K, 2 * WPAD : 2 * WPAD + 4] = (
        es.astype(np.float32).view(np.int8).reshape(K, 4)
    )
    consts[:, 2 * WPAD + 4 : 2 * WPAD + 8] = vinit.view(np.int8).reshape(P2, 4)

    ehat_full = np.exp(em)  # [B, T, K] fp32

    in_maps = []
    for core in range(NCORES):
        eh_local = ehat_full[core * BL : (core + 1) * BL]
        s0, s1 = _host_slabs(eh_local)
        in_maps.append(
            {"slab0": s0, "slab1": s1, "consts": consts}
        )

    res = bass_utils.run_bass_kernel_spmd(
        nc,
        in_maps,
        core_ids=list(range(NCORES)),
        trace=bool(os.environ.get("CRF_TRACE")),
    )
    _cache["last_results"] = res

    # Host assembly of logZ from raw snapshots.
    end_w = np.exp(end).astype(np.float32)
    logz = np.empty(B, np.float32)
    for core in range(NCORES):
        sa_sb = np.asarray(res.results[core]["snaps"]).astype(np.float32)
        fi = np.asarray(res.results[core]["fins"]).astype(np.float32)
        sa = sa_sb[:, : NG * GF]
        sb = sa_sb[:, NG * GF :]

        def chain_slice(arr, c, narrow=False):
            h, q, p = c // 8, (c % 8) // 2, c % 2
            if narrow:
                col0 = (q % 2) * BL
            else:
                col0 = h * 2 * GF + q * BL
            return arr[p * K : (p + 1) * K, col0 : col0 + BL]  # [K, BL]

        acc = np.zeros(BL, np.float64)
        for c in range(S):
            e = chain_slice(fi, c)
            if c == S - 1:
                acc += np.log((e * end_w[:, None]).sum(axis=0))
            else:
                acc += np.log(e.sum(axis=0))
            if c == S - 1:
                st = chain_slice(sb, c, narrow=True)
                acc -= np.log(st.sum(axis=0))
            elif c >= 1:
                st = chain_slice(sa, c)
                acc -= np.log(st.sum(axis=0))
        logz[core * BL : (core + 1) * BL] = acc + (T - 1) * MU

    gold = _gold_score(em, tags, mask, trans, start, end)
    loss = np.mean(logz - gold.astype(np.float64))
    return np.float32(loss)
